# revision 11
# baseline (speedup 1.0000x reference)
"""Trainium2 Bass kernel for nn_AssigmentLayer (8-core data-parallel).

Math (B=131072, T=30, F=10, MAX_LEN=30, K=10 shifts):
  x_c = inputs[:, 0, c] for c in {0,1};  rc_c[m] = x_c[m//30] * w_c[m%30]
  out[b, j, 2i+c] = rc_c[j*B + b - i]   (0 for negative index), i in [0,10)
  out[b, j, 20+t] = inputs[b, j, 2+t],  t in [0,8)

Sharding: batch dim b split contiguously across 8 cores (B8=16384 each).
Per core, for each (j, c), the needed rc values form one contiguous
segment seg[r=2j+c][t] = rc_c[m_base_j + t], m_base_j = j*B + s*B8 - 9.
The host passes index-gathered (no arithmetic) operand streams
  xs[r, t] = x_c[(m_base+t)//30],  ws[r, t] = w_c[(m_base+t)%30]
and the device computes seg = xs * ws (the actual multiplies), keeping
the 60 segment rows resident in SBUF. The 10-shift expansion is 10 PE
transpose-matmuls per 119-row output tile whose lhsT access patterns
are shifted slices of the segment rows; tail features ride along via
strided copies, and full 3360B-contiguous rows DMA out.
"""

import sys

import numpy as np

if "/opt/trn_rl_repo" not in sys.path:
    sys.path.insert(0, "/opt/trn_rl_repo")

B = 131072
T = 30
NCORES = 8
B8 = B // NCORES            # 16384
TILE_P = 128                # output rows per sub-tile (exact tiling)
GRP = 4                     # sub-tiles per group (128 = 32*4)
SEGW = 16464                # segment row width (= 6 * 2744)
NCHUNK = 6
CHW = SEGW // NCHUNK        # 2744

_CACHE = {}


def _sub_tile_starts():
    return [TILE_P * u for u in range(B8 // TILE_P)]  # 128 entries


def _build_nc():
    import concourse.bacc as bacc
    import concourse.tile as tile
    from concourse import mybir
    from contextlib import ExitStack

    f32 = mybir.dt.float32
    nc = bacc.Bacc("TRN2", target_bir_lowering=False, debug=False,
                   num_devices=NCORES)

    tail_in = nc.declare_dram_parameter("tail", [B8, T, 8], f32, isOutput=False)
    xs_in = nc.declare_dram_parameter("xs", [60, SEGW], f32, isOutput=False)
    ws_in = nc.declare_dram_parameter("ws", [60, SEGW], f32, isOutput=False)
    id_in = nc.declare_dram_parameter("ident", [60, 60], f32, isOutput=False)
    out_ext = nc.declare_dram_parameter("out", [B8, T, 28], f32, isOutput=True)

    starts = _sub_tile_starts()
    groups = [starts[i:i + GRP] for i in range(0, len(starts), GRP)]

    with tile.TileContext(nc) as tc:
        with ExitStack() as ctx:
            const_pool = ctx.enter_context(tc.tile_pool(name="const", bufs=1))
            seg_pool = ctx.enter_context(tc.tile_pool(name="seg", bufs=1))
            xw_pool = ctx.enter_context(tc.tile_pool(name="xw", bufs=2))
            ps2_pool = ctx.enter_context(
                tc.tile_pool(name="ps2", bufs=8, space="PSUM"))
            out_pool = ctx.enter_context(tc.tile_pool(name="outp", bufs=3))
            tailp = ctx.enter_context(tc.tile_pool(name="tailp", bufs=3))

            ident = const_pool.tile([60, 60], f32)
            nc.scalar.dma_start(ident[:], id_in[:])

            # persistent segment rows: seg[2j+c, t] = rc_c[m_base_j + t]
            segsb = seg_pool.tile([60, SEGW], f32)

            # ---- stage 1: seg = xs * ws (chunked) ----
            for k in range(NCHUNK):
                xt = xw_pool.tile([60, CHW], f32, tag="xt")
                nc.scalar.dma_start(xt[:], xs_in[:, k * CHW:(k + 1) * CHW])
                wt = xw_pool.tile([60, CHW], f32, tag="wt")
                nc.scalar.dma_start(wt[:], ws_in[:, k * CHW:(k + 1) * CHW])
                nc.vector.tensor_mul(
                    segsb[:, k * CHW:(k + 1) * CHW], xt[:], wt[:])

            # ---- stage 2: shift expansion + tail merge + store ----
            for grp in groups:
                ng = len(grp)
                otile = out_pool.tile([128, 840 * GRP], f32, tag="otile")
                tstg = tailp.tile([128, 240 * GRP], f32, tag="tstg")
                src = tail_in[grp[0]:grp[0] + ng * TILE_P]
                src = src.rearrange("(v p) j t -> p v (j t)", v=ng)
                dst = tstg[:, 0:240 * ng].rearrange("p (v f) -> p v f", v=ng)
                nc.gpsimd.dma_start(dst, src)
                for v, b0 in enumerate(grp):
                    psA = ps2_pool.tile([128, 300], f32, tag="ps2")
                    psB = ps2_pool.tile([128, 300], f32, tag="ps2")
                    for i in range(10):
                        ps = psA if i < 5 else psB
                        col = (i % 5) * 60
                        nc.tensor.transpose(
                            ps[:, col:col + 60],
                            segsb[:, b0 + 9 - i: b0 + 9 - i + TILE_P],
                            ident[:],
                        )
                    ovw = otile[:, 840 * v:840 * (v + 1)]
                    od = ovw.rearrange("p (j i c) -> p j i c", j=30, i=14, c=2)
                    srcA = psA[:].rearrange(
                        "p (i j c) -> p j i c", i=5, j=30, c=2)
                    nc.vector.tensor_copy(od[:, :, 0:5, :], srcA)
                    srcB = psB[:].rearrange(
                        "p (i j c) -> p j i c", i=5, j=30, c=2)
                    nc.scalar.copy(od[:, :, 5:10, :], srcB)
                    # tail interleave (alternate engines)
                    ts = tstg[:, 240 * v:240 * (v + 1)]
                    teng = nc.vector.tensor_copy if v % 2 == 0 else \
                        nc.scalar.copy
                    teng(
                        ovw.rearrange("p (j k) -> p j k", j=30)[:, :, 20:28],
                        ts.rearrange("p (j t) -> p j t", j=30),
                    )
                dst = out_ext[grp[0]:grp[0] + ng * TILE_P]
                dst = dst.rearrange("(v p) j k -> p v (j k)", v=ng)
                src = otile[:, 0:840 * ng].rearrange("p (v f) -> p v f", v=ng)
                nc.gpsimd.dma_start(dst, src)

    nc.compile()
    return nc


def _get_nc():
    if "nc" not in _CACHE:
        _CACHE["nc"] = _build_nc()
    return _CACHE["nc"]


def _prep_core(inputs, w1, w2, s):
    """Per-core input map: pure index gathers, no arithmetic."""
    f32 = np.float32
    x01 = inputs[:, 0, 0:2]                     # (B, 2)
    PAD = 2
    NB = SEGW // 30 + 2                         # 550 batches per row
    xpad = np.zeros((PAD + B + NB + 4, 2), dtype=f32)
    xpad[PAD:PAD + B] = x01
    xs = np.empty((60, SEGW), dtype=f32)
    ws = np.empty((60, SEGW), dtype=f32)
    w = [np.asarray(w1, f32).reshape(T), np.asarray(w2, f32).reshape(T)]
    wtiled = [np.tile(w[c], NB + 1) for c in range(2)]
    for c in range(2):
        for j in range(T):
            m_base = j * B + s * B8 - 9
            mb0 = m_base // 30
            o = m_base - 30 * mb0
            r = 2 * j + c
            xs[r] = np.repeat(
                xpad[PAD + mb0:PAD + mb0 + NB, c], 30)[o:o + SEGW]
            ws[r] = wtiled[c][o:o + SEGW]
    tail = np.ascontiguousarray(inputs[s * B8:(s + 1) * B8, :, 2:])
    return {
        "tail": tail,
        "xs": xs,
        "ws": ws,
        "ident": np.eye(60, dtype=f32),
    }


def _run(inputs, w1, w2, trace=False, trace_kwargs=None):
    from concourse.bass_utils import run_bass_kernel_spmd

    nc = _get_nc()
    inputs = np.asarray(inputs, dtype=np.float32)
    in_maps = [_prep_core(inputs, w1, w2, s) for s in range(NCORES)]
    res = run_bass_kernel_spmd(
        nc, in_maps, core_ids=list(range(NCORES)), trace=trace,
        **(trace_kwargs or {}),
    )
    out = np.concatenate(
        [res.results[i]["out"] for i in range(NCORES)], axis=0)
    return out, res


def kernel(inputs, w1, w2):
    return _run(inputs, w1, w2)[0]


# revision 12
# speedup vs baseline: 1.1934x; 1.1934x over previous
"""Trainium2 Bass kernel for nn_AssigmentLayer (8-core data-parallel).

Math (B=131072, T=30, F=10, MAX_LEN=30, K=10 shifts):
  x_c = inputs[:, 0, c] for c in {0,1};  rc_c[m] = x_c[m//30] * w_c[m%30]
  out[b, j, 2i+c] = rc_c[j*B + b - i]   (0 for negative index), i in [0,10)
  out[b, j, 20+t] = inputs[b, j, 2+t],  t in [0,8)

Sharding: batch dim b split contiguously across 8 cores (B8=16384 each).
Per core, for each (j, c), the needed rc values form one contiguous
segment seg[r=2j+c][t] = rc_c[m_base_j + t], m_base_j = j*B + s*B8 - 9.
The host passes index-gathered (no arithmetic) operand streams
  xs[r, t] = x_c[(m_base+t)//30],  ws[r, t] = w_c[(m_base+t)%30]
and the device computes seg = xs * ws (the actual multiplies), keeping
the 60 segment rows resident in SBUF. The 10-shift expansion is 10 PE
transpose-matmuls per 119-row output tile whose lhsT access patterns
are shifted slices of the segment rows; tail features ride along via
strided copies, and full 3360B-contiguous rows DMA out.
"""

import sys

import numpy as np

if "/opt/trn_rl_repo" not in sys.path:
    sys.path.insert(0, "/opt/trn_rl_repo")

B = 131072
T = 30
NCORES = 8
B8 = B // NCORES            # 16384
TILE_P = 128                # output rows per sub-tile (exact tiling)
GRP = 4                     # sub-tiles per group (128 = 32*4)
SEGW = 16464                # segment row width (= 6 * 2744)
NCHUNK = 6
CHW = SEGW // NCHUNK        # 2744

_CACHE = {}


def _sub_tile_starts():
    return [TILE_P * u for u in range(B8 // TILE_P)]  # 128 entries


def _build_nc():
    import concourse.bacc as bacc
    import concourse.tile as tile
    from concourse import mybir
    from contextlib import ExitStack

    f32 = mybir.dt.float32
    nc = bacc.Bacc("TRN2", target_bir_lowering=False, debug=False,
                   num_devices=NCORES)

    tail_in = nc.declare_dram_parameter("tail", [B8, T, 8], f32, isOutput=False)
    xs_in = nc.declare_dram_parameter("xs", [60, SEGW], f32, isOutput=False)
    ws_in = nc.declare_dram_parameter("ws", [60, SEGW], f32, isOutput=False)
    id_in = nc.declare_dram_parameter("ident", [60, 60], f32, isOutput=False)
    out_ext = nc.declare_dram_parameter("out", [B8, T, 28], f32, isOutput=True)

    starts = _sub_tile_starts()
    groups = [starts[i:i + GRP] for i in range(0, len(starts), GRP)]

    with tile.TileContext(nc) as tc:
        with ExitStack() as ctx:
            const_pool = ctx.enter_context(tc.tile_pool(name="const", bufs=1))
            seg_pool = ctx.enter_context(tc.tile_pool(name="seg", bufs=1))
            xw_pool = ctx.enter_context(tc.tile_pool(name="xw", bufs=2))
            ps2_pool = ctx.enter_context(
                tc.tile_pool(name="ps2", bufs=8, space="PSUM"))
            out_pool = ctx.enter_context(tc.tile_pool(name="outp", bufs=3))
            tailp = ctx.enter_context(tc.tile_pool(name="tailp", bufs=3))

            ident = const_pool.tile([60, 60], f32)
            nc.scalar.dma_start(ident[:], id_in[:])

            # persistent segment rows: seg[2j+c, t] = rc_c[m_base_j + t]
            segsb = seg_pool.tile([60, SEGW], f32)

            def emit_chunk(k):
                xt = xw_pool.tile([60, CHW], f32, tag="xt")
                nc.scalar.dma_start(xt[:], xs_in[:, k * CHW:(k + 1) * CHW])
                wt = xw_pool.tile([60, CHW], f32, tag="wt")
                nc.scalar.dma_start(wt[:], ws_in[:, k * CHW:(k + 1) * CHW])
                nc.vector.tensor_mul(
                    segsb[:, k * CHW:(k + 1) * CHW], xt[:], wt[:])

            GR = GRP * TILE_P            # rows per group (512)

            def emit_group(g):
                # rows of this group: b = g*GR + 4*p + v  (p partition, v slot)
                otile = out_pool.tile([128, 840 * GRP], f32, tag="otile")
                tstg = tailp.tile([128, 240 * GRP], f32, tag="tstg")
                src = tail_in[g * GR:(g + 1) * GR]
                src = src.rearrange("(p v) j t -> p v (j t)", v=GRP)
                dst = tstg[:].rearrange("p (v f) -> p v f", v=GRP)
                nc.gpsimd.dma_start(dst, src)
                for v in range(GRP):
                    psA = ps2_pool.tile([128, 300], f32, tag="ps2")
                    psB = ps2_pool.tile([128, 300], f32, tag="ps2")
                    for i in range(10):
                        ps = psA if i < 5 else psB
                        col = (i % 5) * 60
                        base = g * GR + v + 9 - i
                        nc.tensor.transpose(
                            ps[:, col:col + 60],
                            segsb[:, base:base + 4 * 127 + 1:4],
                            ident[:],
                        )
                    ovw = otile[:, 840 * v:840 * (v + 1)]
                    od = ovw.rearrange("p (j i c) -> p j i c", j=30, i=14, c=2)
                    srcA = psA[:].rearrange(
                        "p (i j c) -> p j i c", i=5, j=30, c=2)
                    nc.vector.tensor_copy(od[:, :, 0:5, :], srcA)
                    srcB = psB[:].rearrange(
                        "p (i j c) -> p j i c", i=5, j=30, c=2)
                    nc.scalar.copy(od[:, :, 5:10, :], srcB)
                    # tail interleave (alternate engines)
                    ts = tstg[:, 240 * v:240 * (v + 1)]
                    teng = nc.vector.tensor_copy if v % 2 == 0 else \
                        nc.scalar.copy
                    teng(
                        ovw.rearrange("p (j k) -> p j k", j=30)[:, :, 20:28],
                        ts.rearrange("p (j t) -> p j t", j=30),
                    )
                dst = out_ext[g * GR:(g + 1) * GR]
                dst = dst.rearrange("(p v) j k -> p v (j k)", v=GRP)
                src = otile[:].rearrange("p (v f) -> p v f", v=GRP)
                nc.gpsimd.dma_start(dst, src)

            # emission order interleaves segment chunks with groups so the
            # scheduler can overlap stage 1 with early stage-2 groups
            ngroups = B8 // GR           # 32
            plan = [("c", 0), ("c", 1)]
            done_c = 2
            for g in range(ngroups):
                need = (g * GR + 520) // CHW + 1
                while done_c < min(need + 1, NCHUNK):
                    plan.append(("c", done_c))
                    done_c += 1
                plan.append(("g", g))
            for kind, idx in plan:
                if kind == "c":
                    emit_chunk(idx)
                else:
                    emit_group(idx)

    nc.compile()
    return nc


def _get_nc():
    if "nc" not in _CACHE:
        _CACHE["nc"] = _build_nc()
    return _CACHE["nc"]


def _prep_core(inputs, w1, w2, s):
    """Per-core input map: pure index gathers, no arithmetic."""
    f32 = np.float32
    x01 = inputs[:, 0, 0:2]                     # (B, 2)
    PAD = 2
    NB = SEGW // 30 + 2                         # 550 batches per row
    xpad = np.zeros((PAD + B + NB + 4, 2), dtype=f32)
    xpad[PAD:PAD + B] = x01
    xs = np.empty((60, SEGW), dtype=f32)
    ws = np.empty((60, SEGW), dtype=f32)
    w = [np.asarray(w1, f32).reshape(T), np.asarray(w2, f32).reshape(T)]
    wtiled = [np.tile(w[c], NB + 1) for c in range(2)]
    for c in range(2):
        for j in range(T):
            m_base = j * B + s * B8 - 9
            mb0 = m_base // 30
            o = m_base - 30 * mb0
            r = 2 * j + c
            xs[r] = np.repeat(
                xpad[PAD + mb0:PAD + mb0 + NB, c], 30)[o:o + SEGW]
            ws[r] = wtiled[c][o:o + SEGW]
    tail = np.ascontiguousarray(inputs[s * B8:(s + 1) * B8, :, 2:])
    return {
        "tail": tail,
        "xs": xs,
        "ws": ws,
        "ident": np.eye(60, dtype=f32),
    }


def _run(inputs, w1, w2, trace=False, trace_kwargs=None):
    from concourse.bass_utils import run_bass_kernel_spmd

    nc = _get_nc()
    inputs = np.asarray(inputs, dtype=np.float32)
    in_maps = [_prep_core(inputs, w1, w2, s) for s in range(NCORES)]
    res = run_bass_kernel_spmd(
        nc, in_maps, core_ids=list(range(NCORES)), trace=trace,
        **(trace_kwargs or {}),
    )
    out = np.concatenate(
        [res.results[i]["out"] for i in range(NCORES)], axis=0)
    return out, res


def kernel(inputs, w1, w2):
    return _run(inputs, w1, w2)[0]
